# revision 29
# baseline (speedup 1.0000x reference)
"""Domain-specific BatchNorm (nn_DSBatchNorm) Trainium2 Bass kernel.

Data-parallel over rows across 8 NeuronCores. Per core:
  pass A: segmented per-domain sums/sumsq/counts via bf16 one-hot matmuls
          into PSUM. The bf16 casts of the last NCACHE chunks stay in SBUF
          so pass B can normalize them without re-reading x from HBM.
  tiny AllReduce of the [8, 2F+1] packed stats (bf16 payload: local sums
          are fp32-exact, the 0.4% bf16 rounding of the partials is far
          inside tolerance and halves the collective's processing time)
  table math: A = gamma*inv*nz, B = beta*nz - A*mean_e  (per-domain [8,F]),
          hi/lo bf16 split of [A|B] replicated to PE quadrants with a
          single REP matmul (no serial SBUF->SBUF DMA chain).
  pass B: per row-tile, one merged matmul gathers per-row [A|B] into PSUM
          (hi/lo bf16 split stacked along K reconstructs f32 exactly in
          the accumulator); DVE computes tmp = x*A (bf16); a second matmul
          accumulates ident @ tmp into the B half of the same PSUM bank
          (out = x*A + B); ACT copies the result to SBUF for the store.
          This splits the elementwise work across PE/DVE/ACT so no single
          engine exceeds the DMA budget (the kernel is HBM-bound).

DMA plumbing: x loads ride the sync HWDGE ring, out stores ride the
scalar ring, collective staging rides gpsimd. Pass-B loads are emitted
on the sync queue right after the last pass-A load so the xcp pool
rotation streams them during the AllReduce window.
"""

import sys

if "/opt/trn_rl_repo" not in sys.path:
    sys.path.insert(0, "/opt/trn_rl_repo")

import numpy as np

import concourse.bacc as bacc
import concourse.bass as bass
import concourse.tile as tile
from concourse import mybir
from concourse.bass_utils import run_bass_kernel_spmd

N_CORES = 8
N, F, D = 262144, 512, 8
NS = N // N_CORES  # rows per core
P = 128
T = NS // P  # row-tiles per core
CHUNK = 8  # row-tiles per DMA chunk (2 MB)
NCHUNKS = T // CHUNK
NCACHE = 10  # trailing chunks kept in SBUF as bf16 between passes
UNCACHED = NCHUNKS - NCACHE
EPS = 1e-5
f32 = mybir.dt.float32
bf16 = mybir.dt.bfloat16
i32 = mybir.dt.int32

_CACHE = {}

# test.py can flip this to get a traced run; grading path leaves it False
TRACE = False
LAST_RESULTS = None


def _build():
    AluOp = mybir.AluOpType
    nc = bacc.Bacc(
        "TRN2", target_bir_lowering=False, debug=False, num_devices=N_CORES
    )

    x = nc.dram_tensor("x", [NS, F], f32, kind="ExternalInput")
    yf = nc.dram_tensor("yf", [NS], f32, kind="ExternalInput")
    gamma = nc.dram_tensor("gamma", [D, F], f32, kind="ExternalInput")
    beta = nc.dram_tensor("beta", [D, F], f32, kind="ExternalInput")
    out = nc.dram_tensor("out", [NS, F], f32, kind="ExternalOutput")

    ident_c = nc.inline_tensor(np.eye(P, dtype=np.float32), name="ident_c")
    # REPh[d, m] = 1 iff m % 32 == d; REPl[d, m] = 1 iff m % 32 == d + 8.
    # Two accumulating matmuls place the hi rows at quadrant offsets 0..7
    # and the lo rows at 8..15 (disjoint output rows) without any
    # partition-crossing engine op or SBUF->SBUF DMA chain.
    reph_np = np.zeros((D, P), dtype=np.float32)
    repl_np = np.zeros((D, P), dtype=np.float32)
    for m in range(P):
        if m % 32 < D:
            reph_np[m % 32, m] = 1.0
        elif m % 32 < 2 * D:
            repl_np[m % 32 - D, m] = 1.0
    reph_c = nc.inline_tensor(reph_np, name="reph_c")
    repl_c = nc.inline_tensor(repl_np, name="repl_c")
    # blk8[r, m] = 1 iff r % 8 == m: sums the 8 gathered stat blocks
    # (AllGather + local matmul-reduce instead of AllReduce)
    blk8_np = np.zeros((N_CORES * D, D), dtype=np.float32)
    for r in range(N_CORES * D):
        blk8_np[r, r % D] = 1.0
    blk8_c = nc.inline_tensor(blk8_np, name="blk8_c")

    # p-major row mapping: partition p, tile t <-> row p*T + t. Stats are
    # permutation-invariant and load/store/one-hot all use the same mapping,
    # so this is just a DMA-friendly tiling (16 KB contiguous per partition
    # per chunk).
    x_r = x[:].rearrange("(p t) f -> p t f", t=T)
    out_r = out[:].rearrange("(p t) f -> p t f", t=T)
    y_r = yf[:].rearrange("(p t) -> p t", t=T)

    W = 2 * F + 1

    with tile.TileContext(nc) as tc:
        with (
            tc.tile_pool(name="consts", bufs=1) as consts,
            tc.tile_pool(name="tables", bufs=1) as tables,
            tc.tile_pool(name="xc", bufs=3) as xcp,
            tc.tile_pool(name="xb", bufs=4) as xbp,
            tc.tile_pool(name="xsq", bufs=4) as xsqp,
            tc.tile_pool(name="oh", bufs=2) as ohp,
            tc.tile_pool(name="oc", bufs=3) as ocp,
            tc.tile_pool(name="oh2", bufs=2) as oh2p,
            tc.tile_pool(name="bsb", bufs=3) as bsbp,
            tc.tile_pool(name="dram", bufs=1, space="DRAM") as dram,
        ):
            # ---- constants ----
            ident = consts.tile([P, P], f32)
            nc.sync.dma_start(out=ident, in_=ident_c[:])
            ident_bf = consts.tile([P, P], bf16)
            nc.scalar.copy(ident_bf, ident)
            reph_f = consts.tile([D, P], f32)
            nc.sync.dma_start(out=reph_f, in_=reph_c[:])
            reph_bf = consts.tile([D, P], bf16)
            nc.scalar.copy(reph_bf, reph_f)
            repl_f = consts.tile([D, P], f32)
            nc.sync.dma_start(out=repl_f, in_=repl_c[:])
            repl_bf = consts.tile([D, P], bf16)
            nc.scalar.copy(repl_bf, repl_f)
            blk8_f = consts.tile([N_CORES * D, D], f32)
            nc.sync.dma_start(out=blk8_f, in_=blk8_c[:])
            blk8_bf = consts.tile([N_CORES * D, D], bf16)
            nc.scalar.copy(blk8_bf, blk8_f)
            # iota_cd[p, k*D + d] = d  (pass-A batched one-hot compare)
            iota_cd_i = consts.tile([P, CHUNK * D], i32)
            nc.gpsimd.iota(
                iota_cd_i, pattern=[[0, CHUNK], [1, D]], base=0,
                channel_multiplier=0,
            )
            iota_cd = consts.tile([P, CHUNK * D], f32)
            nc.vector.tensor_copy(out=iota_cd, in_=iota_cd_i)
            # iota32[p, t*32 + ...]: values [0..7, 0..7] then [8..15, 8..15]
            # per tile: positions d and d+8 both match y=d (hi+lo gather),
            # positions 16..31 never match (pad to 32 so lhsT slices are
            # quadrant-aligned at 0/32/64/96)
            iota32_i32 = consts.tile([P, CHUNK * 4 * D], i32)
            nc.gpsimd.iota(
                iota32_i32, pattern=[[0, CHUNK], [D, 2], [0, 2], [1, D]],
                base=0, channel_multiplier=0,
            )
            iota32 = consts.tile([P, CHUNK * 4 * D], f32)
            nc.vector.tensor_copy(out=iota32, in_=iota32_i32)
            gam = consts.tile([D, F], f32)
            nc.sync.dma_start(out=gam, in_=gamma[:])
            bet = consts.tile([D, F], f32)
            nc.sync.dma_start(out=bet, in_=beta[:])
            ones_bf = consts.tile([P, 1], bf16)
            nc.vector.memset(ones_bf, 1.0)
            y_cols = consts.tile([P, T], f32)
            nc.sync.dma_start(out=y_cols, in_=y_r)

            # bf16 copy of the last NCACHE chunks of x, written during pass A
            xcb = consts.tile([P, NCACHE, CHUNK, F], bf16)

            pack_bf = tables.tile([D, W], bf16)

            # ---- pass A: stats + bf16 tail cache ----
            with tc.tile_pool(name="stat_ps", bufs=1, space="PSUM") as statp:
                psum_ss = statp.tile([D, 2, F], f32)
                psum_cnt = statp.tile([D, 1], f32)

                for c in range(NCHUNKS):
                    xc = xcp.tile([P, CHUNK, F], f32)
                    nc.sync.dma_start(
                        out=xc, in_=x_r[:, c * CHUNK : (c + 1) * CHUNK, :]
                    )
                    ci = c - UNCACHED  # >= 0 for cached chunks
                    ysl = y_cols[:, c * CHUNK : (c + 1) * CHUNK]
                    # batched stats one-hot: [P, CHUNK, D]
                    ohs = ohp.tile([P, CHUNK, D], bf16)
                    ybcd = bass.AP(
                        tensor=ysl.tensor, offset=ysl.offset,
                        ap=list(ysl.ap) + [[0, D]],
                    )
                    nc.vector.tensor_tensor(
                        ohs, iota_cd.rearrange("p (k d) -> p k d", d=D), ybcd,
                        AluOp.is_equal,
                    )
                    for k in range(CHUNK):
                        t = c * CHUNK + k
                        if ci >= 0:
                            xb = xcb[:, ci, k, :]
                        else:
                            xb = xbp.tile([P, F], bf16)
                        # casts split ACT/DVE; square runs on DVE in 2x
                        # 16-bit mode (xb*xb, all-bf16 operands), keeping
                        # both engines under the per-chunk DMA budget
                        if k < 5:
                            nc.scalar.copy(xb, xc[:, k, :])
                        else:
                            nc.vector.tensor_copy(out=xb, in_=xc[:, k, :])
                        xsq = xsqp.tile([P, F], bf16)
                        nc.vector.tensor_tensor(xsq, xb, xb, AluOp.mult)
                        first = t == 0
                        last = t == T - 1
                        oh = ohs[:, k, :]
                        nc.tensor.matmul(
                            psum_ss[:, 0, :], oh, xb,
                            start=first, stop=last, skip_group_check=True,
                        )
                        nc.tensor.matmul(
                            psum_ss[:, 1, :], oh, xsq,
                            start=first, stop=last, skip_group_check=True,
                        )
                        nc.tensor.matmul(
                            psum_cnt, oh, ones_bf,
                            start=first, stop=last, skip_group_check=True,
                        )

                # pass-B prefetches: emitted on the sync queue right after
                # the last pass-A load; xcp pool rotation (WAR deps) lets
                # them stream during the collective window
                prefetched = {}
                for c in range(min(2, UNCACHED)):
                    xc = xcp.tile([P, CHUNK, F], f32)
                    nc.sync.dma_start(
                        out=xc, in_=x_r[:, c * CHUNK : (c + 1) * CHUNK, :]
                    )
                    prefetched[c] = xc

                # ---- pack stats (bf16 cast fused into the PSUM copy) ----
                nc.scalar.copy(pack_bf[:, 0:F], psum_ss[:, 0, :])
                nc.scalar.copy(pack_bf[:, F : 2 * F], psum_ss[:, 1, :])
                nc.scalar.copy(pack_bf[:, 2 * F : W], psum_cnt)

            # ---- stats exchange: AllGather + local matmul-reduce ----
            # (the CC-core AllReduce software add was ~36 us; gather moves
            # the same bytes and the 8-block sum is one cheap PE pass)
            cc_in = dram.tile([D, W], bf16)
            cc_out = dram.tile([N_CORES * D, W], bf16)
            nc.gpsimd.dma_start(out=cc_in, in_=pack_bf)
            nc.gpsimd.collective_compute(
                "AllGather",
                AluOp.bypass,
                replica_groups=[list(range(N_CORES))],
                ins=[cc_in.opt()],
                outs=[cc_out.opt()],
            )
            red64 = tables.tile([N_CORES * D, W], bf16)
            nc.gpsimd.dma_start(out=red64, in_=cc_out)
            mean = tables.tile([D, F], f32)
            var = tables.tile([D, F], f32)
            cntf = tables.tile([D, 1], f32)
            safe = tables.tile([D, 1], f32)
            rn = tables.tile([D, 1], f32)
            mb = tables.tile([D, 1], f32)
            omb = tables.tile([D, 1], f32)
            nz = tables.tile([D, 1], f32)
            eps_t = tables.tile([D, 1], f32)
            nc.vector.memset(eps_t, EPS)
            with tc.tile_pool(name="red_ps", bufs=1, space="PSUM") as redp:
                psum_red = redp.tile([D, W], f32)
                for a in range(0, W, F):
                    b = min(a + F, W)
                    nc.tensor.matmul(
                        psum_red[:, a:b], blk8_bf, red64[:, a:b],
                        start=True, stop=True, skip_group_check=True,
                    )
                S = psum_red[:, 0:F]
                Q = psum_red[:, F : 2 * F]
                cnt = psum_red[:, 2 * F : W]

                # ---- table math (all [8, F] / [8, 1]) ----
                nc.vector.tensor_copy(out=cntf, in_=cnt)
                nc.vector.tensor_scalar(safe, cntf, 1.0, None, AluOp.max)
                nc.vector.reciprocal(rn, safe)
                nc.vector.tensor_scalar(mb, cntf, 1.0, None, AluOp.is_gt)
                nc.vector.tensor_scalar(
                    omb, mb, -1.0, 1.0, AluOp.mult, AluOp.add
                )
                nc.vector.tensor_scalar(nz, cntf, 0.0, None, AluOp.is_gt)
                nc.vector.tensor_scalar(mean, S, rn, None, AluOp.mult)
                nc.vector.tensor_scalar(var, Q, rn, None, AluOp.mult)
            m2 = tables.tile([D, F], f32)
            nc.vector.tensor_tensor(m2, mean, mean, AluOp.mult)
            nc.vector.tensor_tensor(var, var, m2, AluOp.subtract)
            # inv0 = 1/sqrt(var + eps); blend to 1 where count <= 1.
            # approx reciprocal is ~18-bit accurate (vs 2e-2 tolerance) and
            # ~5x faster than the exact DVE reciprocal on [D, F].
            # m2 is dead after the var subtract, reuse it for sd.
            sd = m2
            nc.scalar.activation(
                sd, var, mybir.ActivationFunctionType.Sqrt,
                bias=eps_t[:, 0:1],
            )
            inv = tables.tile([D, F], f32)
            nc.vector.reciprocal_approx_fast(out=inv, in_=sd)
            nc.vector.tensor_scalar(inv, inv, mb, omb, AluOp.mult, AluOp.add)
            # AB = [A | B]: A = gamma*inv*nz, B = beta*nz - A*mean_e
            AB = tables.tile([D, 2 * F], f32)
            A = AB[:, 0:F]
            B = AB[:, F : 2 * F]
            nc.vector.scalar_tensor_tensor(A, gam, nz, inv, AluOp.mult, AluOp.mult)
            me = tables.tile([D, F], f32)
            nc.vector.tensor_scalar(me, mean, mb, None, AluOp.mult)  # mean_e
            nc.vector.tensor_tensor(me, A, me, AluOp.mult)  # A * mean_e
            nc.vector.scalar_tensor_tensor(
                B, bet, nz, me, AluOp.mult, AluOp.subtract
            )

            # ---- hi/lo bf16 split of [A|B], quadrant-replicated ----
            # hi = bf16(AB); lo = bf16(AB - f32(hi)); both stay on
            # partitions 0..7, the REP matmuls move them to the quadrants
            hi_bf = tables.tile([D, 2 * F], bf16)
            hi32 = tables.tile([D, 2 * F], f32)
            lo_bf = tables.tile([D, 2 * F], bf16)
            nc.scalar.copy(hi_bf, AB)
            nc.scalar.copy(hi32, hi_bf)
            nc.vector.tensor_tensor(lo_bf, AB, hi32, AluOp.subtract)
            ABHL = tables.tile([P, 2 * F], bf16)
            with tc.tile_pool(name="rep_ps", bufs=1, space="PSUM") as repp:
                pR = repp.tile([P, 2 * F], f32)
                # matmul dests cannot span PSUM banks: N <= 512 fp32 each
                for half in range(2):
                    sl = slice(half * F, (half + 1) * F)
                    nc.tensor.matmul(
                        pR[:, sl], reph_bf, hi_bf[:, sl],
                        start=True, stop=False, skip_group_check=True,
                    )
                    nc.tensor.matmul(
                        pR[:, sl], repl_bf, lo_bf[:, sl],
                        start=False, stop=True, skip_group_check=True,
                    )
                nc.scalar.copy(ABHL, pR)

            # ---- pass B: normalize ----
            # uncached (HBM re-read) interleaved with cached so the DMA
            # rings and engines stay jointly busy
            cached = list(range(UNCACHED, NCHUNKS))
            uncached = list(range(2, UNCACHED))
            order = list(range(min(2, UNCACHED)))
            stride = max(1, len(uncached) // max(1, len(cached)))
            while cached or uncached:
                take = uncached[:stride]
                del uncached[:stride]
                order.extend(take)
                if cached:
                    order.append(cached.pop(0))
            with (
                tc.tile_pool(name="pAB", bufs=3, space="PSUM") as pABp,
                tc.tile_pool(name="pT", bufs=2, space="PSUM") as pTp,
                tc.tile_pool(name="ohT", bufs=4) as ohTp,
            ):
                for c in order:
                    ci = c - UNCACHED
                    if ci >= 0:
                        xsrc = xcb[:, ci, :, :]
                    elif c in prefetched:
                        xsrc = prefetched.pop(c)
                    else:
                        xc = xcp.tile([P, CHUNK, F], f32)
                        nc.sync.dma_start(
                            out=xc, in_=x_r[:, c * CHUNK : (c + 1) * CHUNK, :]
                        )
                        xsrc = xc
                    # doubled padded one-hot for this chunk + PE transposes
                    ohs2 = oh2p.tile([P, CHUNK * 4 * D], bf16)
                    ysl = y_cols[:, c * CHUNK : (c + 1) * CHUNK]
                    ybc = bass.AP(
                        tensor=ysl.tensor, offset=ysl.offset,
                        ap=list(ysl.ap) + [[0, 4 * D]],
                    )
                    nc.vector.tensor_tensor(
                        ohs2.rearrange("p (k r) -> p k r", r=4 * D),
                        iota32.rearrange("p (k r) -> p k r", r=4 * D),
                        ybc,
                        AluOp.is_equal,
                    )
                    ohTs = []
                    for h in range(CHUNK // 4):
                        psum_oT = pTp.tile([P, P], f32)
                        nc.tensor.matmul(
                            psum_oT,
                            ohs2[:, h * P : (h + 1) * P],
                            ident_bf,
                            start=True, stop=True, skip_group_check=True,
                        )
                        ohT = ohTp.tile([P, P], bf16)
                        nc.scalar.copy(ohT, psum_oT)
                        ohTs.append(ohT)
                    # half-chunk output staging (1 MB stores); one pool
                    # name rotating through 3 bufs
                    ocs = [None, None]
                    # per tile: PE gathers [A|B] into PSUM (two N=512
                    # matmuls), DVE writes oc = x*A, then oc += B in place
                    # (alternating DVE/gpsimd so neither exceeds the DMA
                    # budget). The PE queue holds only gathers/transposes,
                    # so it streams without waiting on the other engines.
                    for k in range(CHUNK):
                        h, l = divmod(k, 4)
                        lhs = ohTs[h][l * 32 : (l + 1) * 32, :]
                        pAB = pABp.tile([P, 2 * F], f32)
                        nc.tensor.matmul(
                            pAB[:, 0:F], lhs,
                            ABHL[l * 32 : (l + 1) * 32, 0:F],
                            start=True, stop=True, skip_group_check=True,
                            tile_position=(l * 32, 0),
                        )
                        nc.tensor.matmul(
                            pAB[:, F : 2 * F], lhs,
                            ABHL[l * 32 : (l + 1) * 32, F : 2 * F],
                            start=True, stop=True, skip_group_check=True,
                            tile_position=(l * 32, 0),
                        )
                        h2, k2 = divmod(k, CHUNK // 2)
                        if k2 == 0:
                            ocs[h2] = ocp.tile(
                                [P, CHUNK // 2, F], f32, name="oc"
                            )
                        ock = ocs[h2][:, k2, :]
                        nc.vector.tensor_tensor(
                            ock, xsrc[:, k, :], pAB[:, 0:F], AluOp.mult
                        )
                        if k % 2 == 0:
                            nc.vector.tensor_tensor(
                                ock, ock, pAB[:, F : 2 * F], AluOp.add
                            )
                        else:
                            # gpsimd cannot read PSUM: ACT (idle here)
                            # stages B_rows to SBUF bf16 first
                            bsb = bsbp.tile([P, F], bf16)
                            nc.scalar.copy(bsb, pAB[:, F : 2 * F])
                            nc.gpsimd.tensor_tensor(
                                ock, ock, bsb, AluOp.add
                            )
                        if k2 == CHUNK // 2 - 1:
                            base = c * CHUNK + h2 * (CHUNK // 2)
                            nc.scalar.dma_start(
                                out=out_r[:, base : base + CHUNK // 2, :],
                                in_=ocs[h2],
                            )

    nc.finalize()
    return nc


def _get_nc():
    if "nc" not in _CACHE:
        _CACHE["nc"] = _build()
    return _CACHE["nc"]


def kernel(x, y, gamma, beta):
    global LAST_RESULTS
    x = np.ascontiguousarray(np.asarray(x), dtype=np.float32)
    yf = np.ascontiguousarray(np.asarray(y).astype(np.float32))
    gamma = np.ascontiguousarray(np.asarray(gamma), dtype=np.float32)
    beta = np.ascontiguousarray(np.asarray(beta), dtype=np.float32)

    nc = _get_nc()
    in_maps = [
        {
            "x": x[i * NS : (i + 1) * NS],
            "yf": yf[i * NS : (i + 1) * NS],
            "gamma": gamma,
            "beta": beta,
        }
        for i in range(N_CORES)
    ]
    res = run_bass_kernel_spmd(nc, in_maps, core_ids=list(range(N_CORES)), trace=TRACE)
    LAST_RESULTS = res
    return np.concatenate([res.results[i]["out"] for i in range(N_CORES)], axis=0)
